# revision 1
# baseline (speedup 1.0000x reference)
"""Causal dot-product attention (low-rank V) on 8 Trainium2 NeuronCores.

Problem: inputs [B=4, N=4096, E=1024], Wq/Wk/Wvdown [E, D=256], Wvup [D, E].
    Q = x Wq; K = x Wk; S = Q K^T / sqrt(D) (causal); A = softmax(S)
    V = x Wvdown Wvup; out = A V

Sharding: core = (batch, key-parity). Each of the 4 batches is handled by a
pair of cores; core parity c owns the interleaved global key blocks {2j+c}
(128 rows each), which balances the causal work exactly. Each core computes
full Q for its batch, K/V for its key half, and produces the *unnormalized*
attention output O_unnorm[4096, 1024] plus softmax row-sums. The host
combines: out = (O_even + O_odd) / (s_even + s_odd).

In-kernel layout: scores are computed transposed, ST[k, q] = K Q^T, so that
(a) softmax sums over k are a ones-vector matmul, (b) the exp'd tile P[k, q]
is directly the stationary operand of the O = P^T V matmul (no transposes
on-device at all; the host pre-transposes the activations once).
"""

import sys

sys.path.insert(0, "/opt/trn_rl_repo")

import numpy as np

import concourse.bacc as bacc
import concourse.mybir as mybir
import concourse.tile as tile

F32 = mybir.dt.float32
F32R = mybir.dt.float32r
BF16 = mybir.dt.bfloat16

B, N, E, D = 4, 4096, 1024, 256
NCORES = 8
KLOC = N // 2  # local keys per core
NKB = KLOC // 128  # 16 local key blocks
NQC = N // 512  # 8 query chunks of 512
NKC = KLOC // 512  # 4 local key chunks of 512
SCALE = 1.0 / np.sqrt(np.float32(D))  # 1/16

_CACHE = {}


def _r(ap):
    """View an fp32 AP as float32r so the PE runs at full (1 cyc/row) rate."""
    return ap.bitcast(F32R)


def _build_nc(reps=1):
    nc = bacc.Bacc("TRN2", target_bir_lowering=False)

    xT = nc.dram_tensor("xT", [E, N], F32R, kind="ExternalInput")
    xkT = nc.dram_tensor("xkT", [E, KLOC], F32R, kind="ExternalInput")
    wq = nc.dram_tensor("wq", [E, D], F32R, kind="ExternalInput")
    wk = nc.dram_tensor("wk", [E, D], F32R, kind="ExternalInput")
    wvd = nc.dram_tensor("wvd", [E, D], F32R, kind="ExternalInput")
    wvu = nc.dram_tensor("wvu", [D, E], F32R, kind="ExternalInput")
    mka = nc.dram_tensor("mka", [128, 512], BF16, kind="ExternalInput")
    mkb = nc.dram_tensor("mkb", [128, 512], BF16, kind="ExternalInput")

    o = nc.dram_tensor("o", [N, E], F32, kind="ExternalOutput")
    ssum = nc.dram_tensor("ssum", [NQC, 512], F32, kind="ExternalOutput")

    with tile.TileContext(nc) as tc:
      for _rep in range(reps):
        with (
            tc.tile_pool(name=f"res{_rep}", bufs=1) as res,
            tc.tile_pool(name=f"consts{_rep}", bufs=1) as consts,
        ):
            # Resident results of the projection phase.
            qt = [res.tile([128, N], F32R, tag=f"qt{d}", name=f"qt{d}") for d in range(2)]
            kt = [res.tile([128, KLOC], F32R, tag=f"kt{d}", name=f"kt{d}") for d in range(2)]
            vt = [res.tile([128, E], BF16, tag=f"v{kb}", name=f"v{kb}") for kb in range(NKB)]

            ones_f = consts.tile([128, 1], F32, tag="ones_f")
            nc.vector.memset(ones_f, 1.0)
            ones = consts.tile([128, 1], BF16, tag="ones")
            nc.vector.tensor_copy(ones, ones_f)
            mask_a = consts.tile([128, 512], BF16, tag="mka")
            mask_b = consts.tile([128, 512], BF16, tag="mkb")

            # ---------------- projections ----------------
            with (
                tc.tile_pool(name="wpool", bufs=1) as wp,
                tc.tile_pool(name="xstream", bufs=2) as xs,
                tc.tile_pool(name="vdtp", bufs=1) as vdp,
                tc.tile_pool(name="pproj", bufs=4, space="PSUM") as pp,
            ):
                wq_t = [wp.tile([128, D], F32R, tag=f"wq{c}", name=f"wq{c}") for c in range(8)]
                wk_t = [wp.tile([128, D], F32R, tag=f"wk{c}", name=f"wk{c}") for c in range(8)]
                wvd_t = [wp.tile([128, D], F32R, tag=f"wvd{c}", name=f"wvd{c}") for c in range(8)]
                wvu_t = [wp.tile([128, E], F32R, tag=f"wvu{d}", name=f"wvu{d}") for d in range(2)]
                for c in range(8):
                    sl = slice(c * 128, (c + 1) * 128)
                    nc.gpsimd.dma_start(out=wk_t[c], in_=wk[sl, :])
                    nc.gpsimd.dma_start(out=wvd_t[c], in_=wvd[sl, :])
                for c in range(8):
                    sl = slice(c * 128, (c + 1) * 128)
                    nc.gpsimd.dma_start(out=wq_t[c], in_=wq[sl, :])
                for d in range(2):
                    nc.gpsimd.dma_start(
                        out=wvu_t[d], in_=wvu[d * 128 : (d + 1) * 128, :]
                    )
                nc.gpsimd.dma_start(out=mask_a, in_=mka[:, :])
                nc.gpsimd.dma_start(out=mask_b, in_=mkb[:, :])

                vdt = [vdp.tile([128, KLOC], F32R, tag=f"vdt{d}", name=f"vdt{d}") for d in range(2)]

                # Merged streaming loop: iteration i does KT/VdT for key chunk
                # kc=i (first 4 iterations), QT for query chunk qc=i, and V for
                # key blocks 2i, 2i+1. Each 2MB x-tile is loaded as two halves
                # split across the two HWDGE queues (sync + scalar) so DMA
                # stays ahead of the PE everywhere.
                for i in range(NQC):
                    if i < NKC:
                        xk_h = []
                        for h, eng in ((0, nc.scalar), (1, nc.sync)):
                            xkh = xs.tile(
                                [128, 4, 512], F32R, tag="xk", bufs=2, name=f"xk{h}"
                            )
                            eng.dma_start(
                                out=xkh,
                                in_=xkT[
                                    h * 512 : (h + 1) * 512, i * 512 : (i + 1) * 512
                                ].rearrange("(c p) q -> p c q", p=128),
                            )
                            xk_h.append(xkh)
                    xq_h = []
                    for h, eng in ((0, nc.sync), (1, nc.scalar)):
                        xqh = xs.tile(
                            [128, 4, 512], F32R, tag="xq", bufs=3, name=f"xq{h}"
                        )
                        eng.dma_start(
                            out=xqh,
                            in_=xT[
                                h * 512 : (h + 1) * 512, i * 512 : (i + 1) * 512
                            ].rearrange("(c p) q -> p c q", p=128),
                        )
                        xq_h.append(xqh)

                    if i < NKC:
                        for w_t, dst in ((wk_t, kt), (wvd_t, vdt)):
                            for d in range(2):
                                ps = pp.tile([128, 512], F32, tag="ps")
                                dsl = slice(d * 128, (d + 1) * 128)
                                for c in range(8):
                                    nc.tensor.matmul(
                                        ps,
                                        lhsT=(w_t[c][:, dsl]),
                                        rhs=(xk_h[c // 4][:, c % 4, :]),
                                        start=(c == 0),
                                        stop=(c == 7),
                                    )
                                nc.vector.tensor_copy(
                                    dst[d][:, i * 512 : (i + 1) * 512], ps
                                )

                    for d in range(2):
                        ps = pp.tile([128, 512], F32, tag="ps")
                        dsl = slice(d * 128, (d + 1) * 128)
                        for c in range(8):
                            nc.tensor.matmul(
                                ps,
                                lhsT=(wq_t[c][:, dsl]),
                                rhs=(xq_h[c // 4][:, c % 4, :]),
                                start=(c == 0),
                                stop=(c == 7),
                            )
                        nc.vector.tensor_copy(qt[d][:, i * 512 : (i + 1) * 512], ps)

                    for kb in (2 * i, 2 * i + 1):
                        ksl = slice(kb * 128, (kb + 1) * 128)
                        for eh in range(2):
                            ps = pp.tile([128, 512], F32, tag="ps")
                            esl = slice(eh * 512, (eh + 1) * 512)
                            for d in range(2):
                                nc.tensor.matmul(
                                    ps,
                                    lhsT=(vdt[d][:, ksl]),
                                    rhs=(wvu_t[d][:, esl]),
                                    start=(d == 0),
                                    stop=(d == 1),
                                )
                            nc.vector.tensor_copy(vt[kb][:, esl], ps)

            # ---------------- attention ----------------
            with (
                tc.tile_pool(name="ppool", bufs=1) as ppool,
                tc.tile_pool(name="stage", bufs=3) as stage,
                tc.tile_pool(name="ps_sc", bufs=2, space="PSUM") as ps_sc,
                tc.tile_pool(name="ps_sum", bufs=2, space="PSUM") as ps_sum,
                tc.tile_pool(name="ps_o", bufs=3, space="PSUM") as ps_o,
            ):
                for qc in range(NQC):
                    nb = 2 * qc + 2  # local key blocks this query chunk attends to
                    qsl = slice(qc * 512, (qc + 1) * 512)
                    pts = []
                    for kb in range(nb):
                        ksl = slice(kb * 128, (kb + 1) * 128)
                        st = ps_sc.tile([128, 512], F32, tag="st")
                        for d in range(2):
                            nc.tensor.matmul(
                                st,
                                lhsT=(kt[d][:, ksl]),
                                rhs=(qt[d][:, qsl]),
                                start=(d == 0),
                                stop=(d == 1),
                            )
                        pt = ppool.tile([128, 512], BF16, tag=f"p{kb}", name=f"p{kb}")
                        nc.scalar.activation(
                            pt, st, mybir.ActivationFunctionType.Exp, scale=float(SCALE)
                        )
                        # The last two blocks straddle the causal diagonal.
                        if kb == nb - 2:
                            nc.vector.tensor_mul(pt, pt, mask_a)
                        elif kb == nb - 1:
                            nc.vector.tensor_mul(pt, pt, mask_b)
                        pts.append(pt)

                    # softmax denominators: sums[1, q] += 1^T P[k, q]
                    sums = ps_sum.tile([1, 512], F32, tag="sums")
                    for kb in range(nb):
                        nc.tensor.matmul(
                            sums,
                            lhsT=(ones),
                            rhs=(pts[kb]),
                            start=(kb == 0),
                            stop=(kb == nb - 1),
                        )
                    ssb = stage.tile([1, 512], F32, tag="ssb")
                    nc.vector.tensor_copy(ssb, sums)
                    nc.sync.dma_start(out=ssum[qc : qc + 1, :], in_=ssb)

                    # O[q, e] += P[k, q]^T V[k, e]
                    for q4 in range(4):
                        qb = qc * 4 + q4
                        q4sl = slice(q4 * 128, (q4 + 1) * 128)
                        for eh in range(2):
                            esl = slice(eh * 512, (eh + 1) * 512)
                            ops = ps_o.tile([128, 512], F32, tag="ops")
                            for kb in range(nb):
                                nc.tensor.matmul(
                                    ops,
                                    lhsT=(pts[kb][:, q4sl]),
                                    rhs=(vt[kb][:, esl]),
                                    start=(kb == 0),
                                    stop=(kb == nb - 1),
                                )
                            ob = stage.tile([128, 512], F32, tag="ob")
                            nc.vector.tensor_copy(ob, ops)
                            nc.sync.dma_start(
                                out=o[qb * 128 : (qb + 1) * 128, esl], in_=ob
                            )
    nc.finalize()
    return nc


def _get_nc():
    if "nc" not in _CACHE:
        _CACHE["nc"] = _build_nc()
    return _CACHE["nc"]


def _host_masks(parity: int):
    y = np.arange(512)[None, :]
    x = np.arange(128)[:, None]
    import ml_dtypes

    mask_a = (y - x - 128 * parity >= 0).astype(ml_dtypes.bfloat16)
    mask_b = (y - x - 256 - 128 * parity >= 0).astype(ml_dtypes.bfloat16)
    return mask_a, mask_b


def kernel(inputs, Wq, Wk, Wvdown, Wvup):
    from concourse.bass_utils import run_bass_kernel_spmd

    inputs = np.asarray(inputs, dtype=np.float32)
    Wq = np.ascontiguousarray(np.asarray(Wq, dtype=np.float32))
    Wk = np.ascontiguousarray(np.asarray(Wk, dtype=np.float32))
    Wvdown = np.ascontiguousarray(np.asarray(Wvdown, dtype=np.float32))
    Wvup = np.ascontiguousarray(np.asarray(Wvup, dtype=np.float32))

    nc = _get_nc()

    in_maps = []
    for core in range(NCORES):
        b, parity = core // 2, core % 2
        xb = inputs[b]  # [N, E]
        xT = np.ascontiguousarray(xb.T)  # [E, N]
        xk = np.ascontiguousarray(
            xb.reshape(N // 128, 128, E)[parity::2].reshape(KLOC, E)
        )
        xkT = np.ascontiguousarray(xk.T)  # [E, KLOC]
        mask_a, mask_b = _host_masks(parity)
        in_maps.append(
            {
                "xT": xT,
                "xkT": xkT,
                "wq": Wq,
                "wk": Wk,
                "wvd": Wvdown,
                "wvu": Wvup,
                "mka": mask_a,
                "mkb": mask_b,
            }
        )

    res = run_bass_kernel_spmd(nc, in_maps, core_ids=list(range(NCORES)))
    results = res.results

    out = np.empty((B, N, E), dtype=np.float32)
    for b in range(B):
        o_sum = results[2 * b]["o"] + results[2 * b + 1]["o"]
        s_sum = (results[2 * b]["ssum"] + results[2 * b + 1]["ssum"]).reshape(N)
        out[b] = o_sum / s_sum[:, None]
    return out



# revision 9
# speedup vs baseline: 1.4944x; 1.4944x over previous
"""Causal dot-product attention (low-rank V) on 8 Trainium2 NeuronCores.

Problem: inputs [B=4, N=4096, E=1024], Wq/Wk/Wvdown [E, D=256], Wvup [D, E].
    Q = x Wq; K = x Wk; S = Q K^T / sqrt(D) (causal); A = softmax(S)
    V = x Wvdown Wvup; out = A V

Key algebraic move vs the previous version: Wvup commutes with the attention
sum AND with the softmax row-normalization, so the big output matmul runs at
rank D=256 instead of E=1024:
    U = P^T Z   with Z = x Wvdown   [q, 256]      (P = exp'd masked scores)
    O = diag(1/rowsum) U Wvup                      (4x fewer PE cycles)

Sharding: core = (batch, query-parity). Core parity c owns the interleaved
global query blocks {2j+c} (128 rows each), which balances causal work. Each
core computes K^T and Z for ALL keys of its batch (duplicated within the
pair; no cross-core comm available), Q^T only for its owned (packed)
queries, then scores/softmax/U/final for its queries. Output rows are owned
exclusively -> normalization happens on device, host only re-interleaves.

All matmul operands are bf16 (f32 PSUM accumulation); scores layout is
ST[k, q] so the softmax denominator is a ones-vector matmul and P feeds the
U matmul without any transpose. Row-sums [1,512] are flipped into
per-partition layout [128,4] with four tiny (free=1) matmuls so the final
PSUM->SBUF copy applies 1/rowsum as a per-partition activation scale.

The program is identical on all cores (SPMD); parity enters only through
per-core inputs: the packed-query activations xqT and 8 causal masks.
"""

import sys

sys.path.insert(0, "/opt/trn_rl_repo")

import numpy as np

import concourse.bacc as bacc
import concourse.mybir as mybir
import concourse.tile as tile

F32 = mybir.dt.float32
F32R = mybir.dt.float32r
BF16 = mybir.dt.bfloat16

B, N, E, D = 4, 4096, 1024, 256
NCORES = 8
QLOC = N // 2          # packed owned queries per core
NCHUNK = N // 512      # 8 streaming x chunks (512 keys each)
NG = QLOC // 512       # 4 query groups of 512 packed queries
SCALE = 1.0 / np.sqrt(np.float32(D))  # 1/16
# unified per-parity-safe free-range starts for the 8 "recent" key blocks
QS = [0, 0, 128, 128, 256, 256, 384, 384]

_CACHE = {}


def _build_nc():
    nc = bacc.Bacc("TRN2", target_bir_lowering=False)

    xT = nc.dram_tensor("xT", [E, N], BF16, kind="ExternalInput")
    xqT = nc.dram_tensor("xqT", [E, QLOC], BF16, kind="ExternalInput")
    wq = nc.dram_tensor("wq", [E, D], BF16, kind="ExternalInput")
    wk = nc.dram_tensor("wk", [E, D], BF16, kind="ExternalInput")
    wvd = nc.dram_tensor("wvd", [E, D], BF16, kind="ExternalInput")
    wvu = nc.dram_tensor("wvu", [D, E], BF16, kind="ExternalInput")
    mk = nc.dram_tensor("mk", [8, 128, 512], BF16, kind="ExternalInput")

    o = nc.dram_tensor("o", [QLOC, E], F32, kind="ExternalOutput")

    with tile.TileContext(nc) as tc:
        with (
            tc.tile_pool(name="res", bufs=1) as res,
            tc.tile_pool(name="consts", bufs=1) as consts,
        ):
            # Resident projection results (all bf16).
            qt = [res.tile([128, QLOC], BF16, tag=f"qt{d}", name=f"qt{d}") for d in range(2)]
            kt = [res.tile([128, N], BF16, tag=f"kt{d}", name=f"kt{d}") for d in range(2)]
            zt = [res.tile([128, D], BF16, tag=f"z{kb}", name=f"z{kb}") for kb in range(N // 128)]

            ones_f = consts.tile([128, 1], F32, tag="ones_f")
            nc.vector.memset(ones_f, 1.0)
            ones = consts.tile([128, 1], BF16, tag="ones")
            nc.vector.tensor_copy(ones, ones_f)
            one1 = consts.tile([1, 1], F32, tag="one1")
            nc.vector.memset(one1, 1.0)
            masks = [
                consts.tile([128, 512], BF16, tag=f"mk{r}", name=f"mk{r}")
                for r in range(8)
            ]
            wvu_t = [
                consts.tile([128, E], BF16, tag=f"wvu{d}", name=f"wvu{d}")
                for d in range(2)
            ]

            # ---------------- projections ----------------
            with (
                tc.tile_pool(name="wpool", bufs=1) as wp,
                tc.tile_pool(name="xstream", bufs=2) as xs,
                tc.tile_pool(name="pproj", bufs=2, space="PSUM") as pp,
            ):
                wq_t = [wp.tile([128, D], BF16, tag=f"wq{cc}", name=f"wq{cc}") for cc in range(8)]
                wk_t = [wp.tile([128, D], BF16, tag=f"wk{cc}", name=f"wk{cc}") for cc in range(8)]
                wvd_t = [wp.tile([128, D], BF16, tag=f"wvd{cc}", name=f"wvd{cc}") for cc in range(8)]
                for cc in range(8):
                    sl = slice(cc * 128, (cc + 1) * 128)
                    nc.gpsimd.dma_start(out=wk_t[cc], in_=wk[sl, :])
                    nc.gpsimd.dma_start(out=wvd_t[cc], in_=wvd[sl, :])
                    nc.gpsimd.dma_start(out=wq_t[cc], in_=wq[sl, :])
                for d in range(2):
                    nc.gpsimd.dma_start(out=wvu_t[d], in_=wvu[d * 128 : (d + 1) * 128, :])
                for r in range(8):
                    nc.gpsimd.dma_start(out=masks[r], in_=mk[r])

                for i in range(NCHUNK):
                    ksl = slice(i * 512, (i + 1) * 512)
                    qsl = slice(i * 256, (i + 1) * 256)
                    xc, xqc = [], []
                    for cc in range(8):
                        eng = nc.sync if cc % 2 == 0 else nc.scalar
                        t = xs.tile([128, 512], BF16, tag=f"xc{cc}", bufs=2, name=f"xc{cc}")
                        eng.dma_start(out=t, in_=xT[cc * 128 : (cc + 1) * 128, ksl])
                        xc.append(t)
                        tq = xs.tile([128, 256], BF16, tag=f"xq{cc}", bufs=2, name=f"xq{cc}")
                        eng.dma_start(out=tq, in_=xqT[cc * 128 : (cc + 1) * 128, qsl])
                        xqc.append(tq)

                    # K^T[d, k] accumulation over E chunks
                    for d in range(2):
                        dsl = slice(d * 128, (d + 1) * 128)
                        ps = pp.tile([128, 512], F32, tag="psk")
                        for cc in range(8):
                            nc.tensor.matmul(
                                ps, lhsT=wk_t[cc][:, dsl], rhs=xc[cc],
                                start=(cc == 0), stop=(cc == 7),
                            )
                        nc.vector.tensor_copy(kt[d][:, ksl], ps)
                    # Z[k, d] per key block (stationary = x chunk slice)
                    for j in range(4):
                        kb = 4 * i + j
                        ps = pp.tile([128, D], F32, tag="psz")
                        for cc in range(8):
                            nc.tensor.matmul(
                                ps, lhsT=xc[cc][:, j * 128 : (j + 1) * 128],
                                rhs=wvd_t[cc], start=(cc == 0), stop=(cc == 7),
                            )
                        nc.vector.tensor_copy(zt[kb], ps)
                    # Q^T[d, packed q]
                    for d in range(2):
                        dsl = slice(d * 128, (d + 1) * 128)
                        ps = pp.tile([128, 256], F32, tag="psq")
                        for cc in range(8):
                            nc.tensor.matmul(
                                ps, lhsT=wq_t[cc][:, dsl], rhs=xqc[cc],
                                start=(cc == 0), stop=(cc == 7),
                            )
                        nc.vector.tensor_copy(qt[d][:, qsl], ps)

            # ---------------- attention ----------------
            with (
                tc.tile_pool(name="ppool", bufs=3) as ppool,
                tc.tile_pool(name="upool", bufs=2) as upool,
                tc.tile_pool(name="stage", bufs=3) as stage,
                tc.tile_pool(name="ps_sc", bufs=2, space="PSUM") as ps_sc,
                tc.tile_pool(name="ps_u", bufs=1, space="PSUM") as ps_u,
                tc.tile_pool(name="ps_sum", bufs=1, space="PSUM") as ps_sum,
                tc.tile_pool(name="ps_rc", bufs=1, space="PSUM") as ps_rc,
                tc.tile_pool(name="ps_o", bufs=2, space="PSUM") as ps_o,
            ):
                for g in range(NG):
                    nkb = 8 * g + 8  # key blocks this group touches (both parities)
                    gq = slice(g * 512, (g + 1) * 512)
                    pu = [
                        ps_u.tile([128, 512], F32, tag=f"pu{d}", name=f"pu{d}")
                        for d in range(2)
                    ]
                    psums = ps_sum.tile([1, 512], F32, tag="psums")
                    for kb in range(nkb):
                        r = kb - 8 * g
                        qs = QS[r] if r >= 0 else 0
                        st = ps_sc.tile([128, 512], F32, tag="st")
                        for d in range(2):
                            nc.tensor.matmul(
                                st[:, qs:],
                                lhsT=kt[d][:, kb * 128 : (kb + 1) * 128],
                                rhs=qt[d][:, g * 512 + qs : (g + 1) * 512],
                                start=(d == 0), stop=(d == 1),
                            )
                        pt = ppool.tile([128, 512], BF16, tag="pt")
                        nc.scalar.activation(
                            pt[:, qs:], st[:, qs:],
                            mybir.ActivationFunctionType.Exp, scale=float(SCALE),
                        )
                        if r >= 0:
                            nc.vector.tensor_mul(pt[:, qs:], pt[:, qs:], masks[r][:, qs:])
                        for d in range(2):
                            nc.tensor.matmul(
                                pu[d][:, qs:],
                                lhsT=zt[kb][:, d * 128 : (d + 1) * 128],
                                rhs=pt[:, qs:],
                                start=(kb == 0), stop=(kb == nkb - 1),
                            )
                        nc.tensor.matmul(
                            psums[:, qs:], lhsT=ones, rhs=pt[:, qs:],
                            start=(kb == 0), stop=(kb == nkb - 1),
                        )

                    ut = [
                        upool.tile([128, 512], BF16, tag=f"ut{d}", name=f"ut{d}")
                        for d in range(2)
                    ]
                    for d in range(2):
                        nc.vector.tensor_copy(ut[d], pu[d])
                    ssb = stage.tile([1, 512], F32, tag="ssb")
                    nc.scalar.copy(ssb, psums)
                    # flip row-sums into per-partition layout [128, 4]
                    prc = ps_rc.tile([128, 4], F32, tag="prc")
                    for s in range(4):
                        nc.tensor.matmul(
                            prc[:, s : s + 1],
                            lhsT=ssb[:, s * 128 : (s + 1) * 128],
                            rhs=one1,
                            start=True, stop=True,
                        )
                    recg = stage.tile([128, 4], F32, tag="recg")
                    nc.vector.reciprocal(recg, prc)

                    for s in range(4):
                        for eh in range(2):
                            esl = slice(eh * 512, (eh + 1) * 512)
                            po = ps_o.tile([128, 512], F32, tag="po")
                            for d in range(2):
                                nc.tensor.matmul(
                                    po,
                                    lhsT=ut[d][:, s * 128 : (s + 1) * 128],
                                    rhs=wvu_t[d][:, esl],
                                    start=(d == 0), stop=(d == 1),
                                )
                            ob = stage.tile([128, 512], F32, tag="ob")
                            nc.vector.tensor_scalar_mul(ob, po, recg[:, s : s + 1])
                            nc.sync.dma_start(
                                out=o[(4 * g + s) * 128 : (4 * g + s + 1) * 128, esl],
                                in_=ob,
                            )
    nc.finalize()
    return nc


def _get_nc():
    if "nc" not in _CACHE:
        _CACHE["nc"] = _build_nc()
    return _CACHE["nc"]


def _host_masks(parity: int):
    """8 masks [128, 512] for the 8 most-recent key blocks of each group.

    mask[r][p, s*128 + t]: owned query block o = 2s+parity vs key rel-block r:
    o > r -> 1 ; o == r -> (t >= p) ; o < r -> 0.
    """
    import ml_dtypes

    t = np.arange(128)[None, :]
    p = np.arange(128)[:, None]
    tri = (t >= p).astype(np.float32)
    out = np.zeros((8, 128, 512), dtype=np.float32)
    for r in range(8):
        for s in range(4):
            osb = 2 * s + parity
            if osb > r:
                out[r][:, s * 128 : (s + 1) * 128] = 1.0
            elif osb == r:
                out[r][:, s * 128 : (s + 1) * 128] = tri
    return out.astype(ml_dtypes.bfloat16)


def _host_inputs(inputs, Wq, Wk, Wvdown, Wvup):
    import ml_dtypes

    bf16 = ml_dtypes.bfloat16
    inputs = np.asarray(inputs, dtype=np.float32)
    w = {
        "wq": np.ascontiguousarray(np.asarray(Wq, dtype=np.float32)).astype(bf16),
        "wk": np.ascontiguousarray(np.asarray(Wk, dtype=np.float32)).astype(bf16),
        "wvd": np.ascontiguousarray(np.asarray(Wvdown, dtype=np.float32)).astype(bf16),
        "wvu": np.ascontiguousarray(np.asarray(Wvup, dtype=np.float32)).astype(bf16),
    }
    xT_b = [np.ascontiguousarray(inputs[b].T).astype(bf16) for b in range(B)]
    masks = [_host_masks(0), _host_masks(1)]
    in_maps = []
    for core in range(NCORES):
        b, c = core // 2, core % 2
        xq = np.ascontiguousarray(
            xT_b[b].reshape(E, N // 128, 128)[:, c::2, :].reshape(E, QLOC)
        )
        in_maps.append(
            {"xT": xT_b[b], "xqT": xq, "mk": masks[c], **w}
        )
    return in_maps


def _assemble(results):
    out = np.empty((B, N, E), dtype=np.float32)
    for b in range(B):
        for c in range(2):
            ob = results[2 * b + c]["o"]
            out[b].reshape(N // 128, 128, E)[c::2] = ob.reshape(QLOC // 128, 128, E)
    return out


def kernel(inputs, Wq, Wk, Wvdown, Wvup):
    from concourse.bass_utils import run_bass_kernel_spmd

    nc = _get_nc()
    in_maps = _host_inputs(inputs, Wq, Wk, Wvdown, Wvup)
    res = run_bass_kernel_spmd(nc, in_maps, core_ids=list(range(NCORES)))
    return _assemble(res.results)


# revision 18
# speedup vs baseline: 1.6499x; 1.1040x over previous
"""Causal dot-product attention (low-rank V) on 8 Trainium2 NeuronCores.

Problem: inputs [B=4, N=4096, E=1024], Wq/Wk/Wvdown [E, D=256], Wvup [D, E].
    Q = x Wq; K = x Wk; S = Q K^T / sqrt(D) (causal); A = softmax(S)
    V = x Wvdown Wvup; out = A V

Key algebraic move vs the previous version: Wvup commutes with the attention
sum AND with the softmax row-normalization, so the big output matmul runs at
rank D=256 instead of E=1024:
    U = P^T Z   with Z = x Wvdown   [q, 256]      (P = exp'd masked scores)
    O = diag(1/rowsum) U Wvup                      (4x fewer PE cycles)

Sharding: core = (batch, query-parity). Core parity c owns the interleaved
global query blocks {2j+c} (128 rows each), which balances causal work. Each
core computes K^T and Z for ALL keys of its batch (duplicated within the
pair; no cross-core comm available), Q^T only for its owned (packed)
queries, then scores/softmax/U/final for its queries. Output rows are owned
exclusively -> normalization happens on device, host only re-interleaves.

All matmul operands are bf16 (f32 PSUM accumulation); scores layout is
ST[k, q] so the softmax denominator is a ones-vector matmul and P feeds the
U matmul without any transpose. Row-sums [1,512] are flipped into
per-partition layout [128,4] with four tiny (free=1) matmuls so the final
PSUM->SBUF copy applies 1/rowsum as a per-partition activation scale.

The program is identical on all cores (SPMD); parity enters only through
per-core inputs: the packed-query activations xqT and 8 causal masks.
"""

import sys

sys.path.insert(0, "/opt/trn_rl_repo")

import numpy as np

import concourse.bacc as bacc
import concourse.mybir as mybir
import concourse.tile as tile

F32 = mybir.dt.float32
F32R = mybir.dt.float32r
BF16 = mybir.dt.bfloat16

B, N, E, D = 4, 4096, 1024, 256
NCORES = 8
QLOC = N // 2          # packed owned queries per core
NCHUNK = N // 512      # 8 streaming x chunks (512 keys each)
NG = QLOC // 512       # 4 query groups of 512 packed queries
SCALE = 1.0 / np.sqrt(np.float32(D))  # 1/16
# unified per-parity-safe free-range starts for the 8 "recent" key blocks
QS = [0, 0, 128, 128, 256, 256, 384, 384]

_CACHE = {}


def _build_nc():
    nc = bacc.Bacc("TRN2", target_bir_lowering=False)

    xT = nc.dram_tensor("xT", [E, N], BF16, kind="ExternalInput")
    xqT = nc.dram_tensor("xqT", [E, QLOC], BF16, kind="ExternalInput")
    wq = nc.dram_tensor("wq", [E, D], BF16, kind="ExternalInput")
    wk = nc.dram_tensor("wk", [E, D], BF16, kind="ExternalInput")
    wvd = nc.dram_tensor("wvd", [E, D], BF16, kind="ExternalInput")
    wvu = nc.dram_tensor("wvu", [D, E], BF16, kind="ExternalInput")
    mk = nc.dram_tensor("mk", [8, 128, 512], BF16, kind="ExternalInput")

    o = nc.dram_tensor("o", [QLOC, E], F32, kind="ExternalOutput")

    with tile.TileContext(nc) as tc:
        with (
            tc.tile_pool(name="res", bufs=1) as res,
            tc.tile_pool(name="consts", bufs=1) as consts,
        ):
            # Resident projection results (all bf16).
            qt = [res.tile([128, QLOC], BF16, tag=f"qt{d}", name=f"qt{d}") for d in range(2)]
            kt = [res.tile([128, N], BF16, tag=f"kt{d}", name=f"kt{d}") for d in range(2)]
            zt = [res.tile([128, D], BF16, tag=f"z{kb}", name=f"z{kb}") for kb in range(N // 128)]

            ones_f = consts.tile([128, 128], F32, tag="ones_f")
            nc.vector.memset(ones_f, 1.0)
            # all-ones stationary: the sums matmul runs M=128 (full column
            # groups, 215ns) instead of M=1 (307ns); every output row holds
            # the same column sums.
            ones = consts.tile([128, 128], BF16, tag="ones")
            nc.vector.tensor_copy(ones, ones_f)
            # 1/128 moving operand for the flip matmul: contracting the 128
            # identical sum rows against it yields the sums again.
            inv128 = consts.tile([128, 1], F32, tag="inv128")
            nc.vector.memset(inv128, 1.0 / 128.0)
            masks = [
                consts.tile([128, 512], BF16, tag=f"mk{r}", name=f"mk{r}")
                for r in range(8)
            ]
            wvu_t = [
                consts.tile([128, E], BF16, tag=f"wvu{d}", name=f"wvu{d}")
                for d in range(2)
            ]

            # ---------------- projections ----------------
            with (
                tc.tile_pool(name="wpool", bufs=1) as wp,
                tc.tile_pool(name="xstream", bufs=2) as xs,
                tc.tile_pool(name="pproj", bufs=2, space="PSUM") as pp,
            ):
                wq_t = [wp.tile([128, D], BF16, tag=f"wq{cc}", name=f"wq{cc}") for cc in range(8)]
                wk_t = [wp.tile([128, D], BF16, tag=f"wk{cc}", name=f"wk{cc}") for cc in range(8)]
                wvd_t = [wp.tile([128, D], BF16, tag=f"wvd{cc}", name=f"wvd{cc}") for cc in range(8)]
                for cc in range(8):
                    sl = slice(cc * 128, (cc + 1) * 128)
                    nc.gpsimd.dma_start(out=wk_t[cc], in_=wk[sl, :])
                    nc.gpsimd.dma_start(out=wvd_t[cc], in_=wvd[sl, :])
                    nc.gpsimd.dma_start(out=wq_t[cc], in_=wq[sl, :])
                for d in range(2):
                    nc.gpsimd.dma_start(out=wvu_t[d], in_=wvu[d * 128 : (d + 1) * 128, :])
                for r in range(8):
                    nc.gpsimd.dma_start(out=masks[r], in_=mk[r])

                xqc = None
                for i in range(NCHUNK):
                    ksl = slice(i * 512, (i + 1) * 512)
                    xc = []
                    for cc in range(8):
                        eng = nc.sync if cc % 2 == 0 else nc.scalar
                        t = xs.tile([128, 512], BF16, tag=f"xc{cc}", bufs=2, name=f"xc{cc}")
                        eng.dma_start(out=t, in_=xT[cc * 128 : (cc + 1) * 128, ksl])
                        xc.append(t)
                    if i % 2 == 0:
                        qsl = slice(i * 256, i * 256 + 512)
                        xqc = []
                        for cc in range(8):
                            eng = nc.sync if cc % 2 == 0 else nc.scalar
                            tq = xs.tile(
                                [128, 512], BF16, tag=f"xq{cc}", bufs=2, name=f"xq{cc}"
                            )
                            eng.dma_start(
                                out=tq, in_=xqT[cc * 128 : (cc + 1) * 128, qsl]
                            )
                            xqc.append(tq)

                    # K^T[d, k] accumulation over E chunks
                    for d in range(2):
                        dsl = slice(d * 128, (d + 1) * 128)
                        ps = pp.tile([128, 512], F32, tag="psk")
                        for cc in range(8):
                            nc.tensor.matmul(
                                ps, lhsT=wk_t[cc][:, dsl], rhs=xc[cc],
                                start=(cc == 0), stop=(cc == 7),
                            )
                        nc.vector.tensor_copy(kt[d][:, ksl], ps)
                    # Z[k, d] per key block (stationary = x chunk slice)
                    for j in range(4):
                        kb = 4 * i + j
                        ps = pp.tile([128, D], F32, tag="psz")
                        for cc in range(8):
                            nc.tensor.matmul(
                                ps, lhsT=xc[cc][:, j * 128 : (j + 1) * 128],
                                rhs=wvd_t[cc], start=(cc == 0), stop=(cc == 7),
                            )
                        nc.vector.tensor_copy(zt[kb], ps)
                    # Q^T[d, packed q] — free-512, every other chunk
                    if i % 2 == 1:
                        qsl = slice((i - 1) * 256, (i + 1) * 256)
                        for d in range(2):
                            dsl = slice(d * 128, (d + 1) * 128)
                            ps = pp.tile([128, 512], F32, tag="psq")
                            for cc in range(8):
                                nc.tensor.matmul(
                                    ps, lhsT=wq_t[cc][:, dsl], rhs=xqc[cc],
                                    start=(cc == 0), stop=(cc == 7),
                                )
                            nc.vector.tensor_copy(qt[d][:, qsl], ps)

            # ---------------- attention ----------------
            with (
                tc.tile_pool(name="ppool", bufs=4) as ppool,
                tc.tile_pool(name="upool", bufs=2) as upool,
                tc.tile_pool(name="stage", bufs=3) as stage,
                tc.tile_pool(name="ps_sc", bufs=3, space="PSUM") as ps_sc,
                tc.tile_pool(name="ps_u", bufs=1, space="PSUM") as ps_u,
                tc.tile_pool(name="ps_sum", bufs=1, space="PSUM") as ps_sum,
                tc.tile_pool(name="ps_o", bufs=2, space="PSUM") as ps_o,
            ):
                for g in range(NG):
                    nkb = 8 * g + 8  # key blocks this group touches (both parities)
                    pu = [
                        ps_u.tile([128, 512], F32, tag=f"pu{d}", name=f"pu{d}")
                        for d in range(2)
                    ]
                    psums = ps_sum.tile([128, 512], F32, tag="psums")
                    for kb in range(nkb):
                        r = kb - 8 * g
                        qs = QS[r] if r >= 0 else 0
                        st = ps_sc.tile([128, 512], F32, tag="st")
                        for d in range(2):
                            nc.tensor.matmul(
                                st[:, qs:],
                                lhsT=kt[d][:, kb * 128 : (kb + 1) * 128],
                                rhs=qt[d][:, g * 512 + qs : (g + 1) * 512],
                                start=(d == 0), stop=(d == 1),
                            )
                        pt = ppool.tile([128, 512], BF16, tag="pt")
                        nc.scalar.activation(
                            pt[:, qs:], st[:, qs:],
                            mybir.ActivationFunctionType.Exp, scale=float(SCALE),
                        )
                        if r >= 0:
                            nc.vector.tensor_mul(
                                pt[:, qs:], pt[:, qs:], masks[r][:, qs:]
                            )
                        for d in range(2):
                            nc.tensor.matmul(
                                pu[d][:, qs:],
                                lhsT=zt[kb][:, d * 128 : (d + 1) * 128],
                                rhs=pt[:, qs:],
                                start=(kb == 0), stop=(kb == nkb - 1),
                            )
                        nc.tensor.matmul(
                            psums[:, qs:], lhsT=ones, rhs=pt[:, qs:],
                            start=(kb == 0), stop=(kb == nkb - 1),
                        )

                    ut = [
                        upool.tile([128, 512], BF16, tag=f"ut{d}", name=f"ut{d}")
                        for d in range(2)
                    ]
                    for d in range(2):
                        nc.scalar.copy(ut[d], pu[d])
                    # every psums row holds the same column sums; flip them to
                    # per-partition layout [128, 4] with 1/128-scaled matmuls.
                    ssb = stage.tile([128, 512], F32, tag="ssb")
                    nc.vector.tensor_copy(ssb, psums)
                    prc = ps_o.tile([128, 512], F32, tag="po", name="prc")
                    for s in range(4):
                        nc.tensor.matmul(
                            prc[:, s : s + 1],
                            lhsT=ssb[:, s * 128 : (s + 1) * 128],
                            rhs=inv128,
                            start=True, stop=True,
                        )
                    recg = stage.tile([128, 4], F32, tag="recg")
                    nc.vector.reciprocal(recg, prc[:, 0:4])

                    for s in range(4):
                        for eh in range(2):
                            esl = slice(eh * 512, (eh + 1) * 512)
                            po = ps_o.tile([128, 512], F32, tag="po")
                            for d in range(2):
                                nc.tensor.matmul(
                                    po,
                                    lhsT=ut[d][:, s * 128 : (s + 1) * 128],
                                    rhs=wvu_t[d][:, esl],
                                    start=(d == 0), stop=(d == 1),
                                )
                            ob = stage.tile([128, 512], F32, tag="ob")
                            nc.vector.tensor_scalar_mul(ob, po, recg[:, s : s + 1])
                            nc.sync.dma_start(
                                out=o[(4 * g + s) * 128 : (4 * g + s + 1) * 128, esl],
                                in_=ob,
                            )
    nc.finalize()
    return nc


def _get_nc():
    if "nc" not in _CACHE:
        _CACHE["nc"] = _build_nc()
    return _CACHE["nc"]


def _host_masks(parity: int):
    """8 masks [128, 512] for the 8 most-recent key blocks of each group.

    mask[r][p, s*128 + t]: owned query block o = 2s+parity vs key rel-block r:
    o > r -> 1 ; o == r -> (t >= p) ; o < r -> 0.
    """
    import ml_dtypes

    t = np.arange(128)[None, :]
    p = np.arange(128)[:, None]
    tri = (t >= p).astype(np.float32)
    out = np.zeros((8, 128, 512), dtype=np.float32)
    for r in range(8):
        for s in range(4):
            osb = 2 * s + parity
            if osb > r:
                out[r][:, s * 128 : (s + 1) * 128] = 1.0
            elif osb == r:
                out[r][:, s * 128 : (s + 1) * 128] = tri
    return out.astype(ml_dtypes.bfloat16)


def _host_inputs(inputs, Wq, Wk, Wvdown, Wvup):
    import ml_dtypes

    bf16 = ml_dtypes.bfloat16
    inputs = np.asarray(inputs, dtype=np.float32)
    w = {
        "wq": np.ascontiguousarray(np.asarray(Wq, dtype=np.float32)).astype(bf16),
        "wk": np.ascontiguousarray(np.asarray(Wk, dtype=np.float32)).astype(bf16),
        "wvd": np.ascontiguousarray(np.asarray(Wvdown, dtype=np.float32)).astype(bf16),
        "wvu": np.ascontiguousarray(np.asarray(Wvup, dtype=np.float32)).astype(bf16),
    }
    xT_b = [np.ascontiguousarray(inputs[b].T).astype(bf16) for b in range(B)]
    masks = [_host_masks(0), _host_masks(1)]
    in_maps = []
    for core in range(NCORES):
        b, c = core // 2, core % 2
        xq = np.ascontiguousarray(
            xT_b[b].reshape(E, N // 128, 128)[:, c::2, :].reshape(E, QLOC)
        )
        in_maps.append(
            {"xT": xT_b[b], "xqT": xq, "mk": masks[c], **w}
        )
    return in_maps


def _assemble(results):
    out = np.empty((B, N, E), dtype=np.float32)
    for b in range(B):
        for c in range(2):
            ob = results[2 * b + c]["o"]
            out[b].reshape(N // 128, 128, E)[c::2] = ob.reshape(QLOC // 128, 128, E)
    return out


def kernel(inputs, Wq, Wk, Wvdown, Wvup):
    from concourse.bass_utils import run_bass_kernel_spmd

    nc = _get_nc()
    in_maps = _host_inputs(inputs, Wq, Wk, Wvdown, Wvup)
    res = run_bass_kernel_spmd(nc, in_maps, core_ids=list(range(NCORES)))
    return _assemble(res.results)


# revision 19
# speedup vs baseline: 1.6853x; 1.0215x over previous
"""Causal dot-product attention (low-rank V) on 8 Trainium2 NeuronCores.

Problem: inputs [B=4, N=4096, E=1024], Wq/Wk/Wvdown [E, D=256], Wvup [D, E].
    Q = x Wq; K = x Wk; S = Q K^T / sqrt(D) (causal); A = softmax(S)
    V = x Wvdown Wvup; out = A V

Key algebraic move vs the previous version: Wvup commutes with the attention
sum AND with the softmax row-normalization, so the big output matmul runs at
rank D=256 instead of E=1024:
    U = P^T Z   with Z = x Wvdown   [q, 256]      (P = exp'd masked scores)
    O = diag(1/rowsum) U Wvup                      (4x fewer PE cycles)

Sharding: core = (batch, query-parity). Core parity c owns the interleaved
global query blocks {2j+c} (128 rows each), which balances causal work. Each
core computes K^T and Z for ALL keys of its batch (duplicated within the
pair; no cross-core comm available), Q^T only for its owned (packed)
queries, then scores/softmax/U/final for its queries. Output rows are owned
exclusively -> normalization happens on device, host only re-interleaves.

All matmul operands are bf16 (f32 PSUM accumulation); scores layout is
ST[k, q] so the softmax denominator is a ones-vector matmul and P feeds the
U matmul without any transpose. Row-sums [1,512] are flipped into
per-partition layout [128,4] with four tiny (free=1) matmuls so the final
PSUM->SBUF copy applies 1/rowsum as a per-partition activation scale.

The program is identical on all cores (SPMD); parity enters only through
per-core inputs: the packed-query activations xqT and 8 causal masks.
"""

import sys

sys.path.insert(0, "/opt/trn_rl_repo")

import numpy as np

import concourse.bacc as bacc
import concourse.mybir as mybir
import concourse.tile as tile

F32 = mybir.dt.float32
F32R = mybir.dt.float32r
BF16 = mybir.dt.bfloat16

B, N, E, D = 4, 4096, 1024, 256
NCORES = 8
QLOC = N // 2          # packed owned queries per core
NCHUNK = N // 512      # 8 streaming x chunks (512 keys each)
NG = QLOC // 512       # 4 query groups of 512 packed queries
SCALE = 1.0 / np.sqrt(np.float32(D))  # 1/16
# unified per-parity-safe free-range starts for the 8 "recent" key blocks
QS = [0, 0, 128, 128, 256, 256, 384, 384]

_CACHE = {}


def _build_nc():
    nc = bacc.Bacc("TRN2", target_bir_lowering=False)

    xT = nc.dram_tensor("xT", [E, N], BF16, kind="ExternalInput")
    xqT = nc.dram_tensor("xqT", [E, QLOC], BF16, kind="ExternalInput")
    wq = nc.dram_tensor("wq", [E, D], BF16, kind="ExternalInput")
    wk = nc.dram_tensor("wk", [E, D], BF16, kind="ExternalInput")
    wvd = nc.dram_tensor("wvd", [E, D], BF16, kind="ExternalInput")
    wvu = nc.dram_tensor("wvu", [D, E], BF16, kind="ExternalInput")
    mk = nc.dram_tensor("mk", [8, 128, 512], BF16, kind="ExternalInput")

    o = nc.dram_tensor("o", [QLOC, E], BF16, kind="ExternalOutput")

    with tile.TileContext(nc) as tc:
        with (
            tc.tile_pool(name="res", bufs=1) as res,
            tc.tile_pool(name="consts", bufs=1) as consts,
        ):
            # Resident projection results (all bf16).
            qt = [res.tile([128, QLOC], BF16, tag=f"qt{d}", name=f"qt{d}") for d in range(2)]
            kt = [res.tile([128, N], BF16, tag=f"kt{d}", name=f"kt{d}") for d in range(2)]
            zt = [res.tile([128, D], BF16, tag=f"z{kb}", name=f"z{kb}") for kb in range(N // 128)]

            ones_f = consts.tile([128, 128], F32, tag="ones_f")
            nc.vector.memset(ones_f, 1.0)
            # all-ones stationary: the sums matmul runs M=128 (full column
            # groups, 215ns) instead of M=1 (307ns); every output row holds
            # the same column sums.
            ones = consts.tile([128, 128], BF16, tag="ones")
            nc.vector.tensor_copy(ones, ones_f)
            # 1/128 moving operand for the flip matmul: contracting the 128
            # identical sum rows against it yields the sums again.
            inv128 = consts.tile([128, 1], F32, tag="inv128")
            nc.vector.memset(inv128, 1.0 / 128.0)
            masks = [
                consts.tile([128, 512], BF16, tag=f"mk{r}", name=f"mk{r}")
                for r in range(8)
            ]
            wvu_t = [
                consts.tile([128, E], BF16, tag=f"wvu{d}", name=f"wvu{d}")
                for d in range(2)
            ]

            # ---------------- projections ----------------
            with (
                tc.tile_pool(name="wpool", bufs=1) as wp,
                tc.tile_pool(name="xstream", bufs=2) as xs,
                tc.tile_pool(name="pproj", bufs=2, space="PSUM") as pp,
            ):
                wq_t = [wp.tile([128, D], BF16, tag=f"wq{cc}", name=f"wq{cc}") for cc in range(8)]
                wk_t = [wp.tile([128, D], BF16, tag=f"wk{cc}", name=f"wk{cc}") for cc in range(8)]
                wvd_t = [wp.tile([128, D], BF16, tag=f"wvd{cc}", name=f"wvd{cc}") for cc in range(8)]
                for cc in range(8):
                    sl = slice(cc * 128, (cc + 1) * 128)
                    nc.gpsimd.dma_start(out=wk_t[cc], in_=wk[sl, :])
                    nc.gpsimd.dma_start(out=wvd_t[cc], in_=wvd[sl, :])
                    nc.gpsimd.dma_start(out=wq_t[cc], in_=wq[sl, :])
                for d in range(2):
                    nc.gpsimd.dma_start(out=wvu_t[d], in_=wvu[d * 128 : (d + 1) * 128, :])
                for r in range(8):
                    nc.gpsimd.dma_start(out=masks[r], in_=mk[r])

                xqc = None
                for i in range(NCHUNK):
                    ksl = slice(i * 512, (i + 1) * 512)
                    xc = []
                    for cc in range(8):
                        eng = nc.sync if cc % 2 == 0 else nc.scalar
                        t = xs.tile([128, 512], BF16, tag=f"xc{cc}", bufs=2, name=f"xc{cc}")
                        eng.dma_start(out=t, in_=xT[cc * 128 : (cc + 1) * 128, ksl])
                        xc.append(t)
                    if i % 2 == 0:
                        qsl = slice(i * 256, i * 256 + 512)
                        xqc = []
                        for cc in range(8):
                            eng = nc.sync if cc % 2 == 0 else nc.scalar
                            tq = xs.tile(
                                [128, 512], BF16, tag=f"xq{cc}", bufs=2, name=f"xq{cc}"
                            )
                            eng.dma_start(
                                out=tq, in_=xqT[cc * 128 : (cc + 1) * 128, qsl]
                            )
                            xqc.append(tq)

                    # K^T[d, k] accumulation over E chunks
                    for d in range(2):
                        dsl = slice(d * 128, (d + 1) * 128)
                        ps = pp.tile([128, 512], F32, tag="psk")
                        for cc in range(8):
                            nc.tensor.matmul(
                                ps, lhsT=wk_t[cc][:, dsl], rhs=xc[cc],
                                start=(cc == 0), stop=(cc == 7),
                            )
                        nc.vector.tensor_copy(kt[d][:, ksl], ps)
                    # Z[k, d] per key block (stationary = x chunk slice)
                    for j in range(4):
                        kb = 4 * i + j
                        ps = pp.tile([128, D], F32, tag="psz")
                        for cc in range(8):
                            nc.tensor.matmul(
                                ps, lhsT=xc[cc][:, j * 128 : (j + 1) * 128],
                                rhs=wvd_t[cc], start=(cc == 0), stop=(cc == 7),
                            )
                        nc.vector.tensor_copy(zt[kb], ps)
                    # Q^T[d, packed q] — free-512, every other chunk
                    if i % 2 == 1:
                        qsl = slice((i - 1) * 256, (i + 1) * 256)
                        for d in range(2):
                            dsl = slice(d * 128, (d + 1) * 128)
                            ps = pp.tile([128, 512], F32, tag="psq")
                            for cc in range(8):
                                nc.tensor.matmul(
                                    ps, lhsT=wq_t[cc][:, dsl], rhs=xqc[cc],
                                    start=(cc == 0), stop=(cc == 7),
                                )
                            nc.vector.tensor_copy(qt[d][:, qsl], ps)

            # ---------------- attention ----------------
            with (
                tc.tile_pool(name="ppool", bufs=4) as ppool,
                tc.tile_pool(name="upool", bufs=2) as upool,
                tc.tile_pool(name="stage", bufs=3) as stage,
                tc.tile_pool(name="ps_sc", bufs=3, space="PSUM") as ps_sc,
                tc.tile_pool(name="ps_u", bufs=1, space="PSUM") as ps_u,
                tc.tile_pool(name="ps_sum", bufs=1, space="PSUM") as ps_sum,
                tc.tile_pool(name="ps_o", bufs=2, space="PSUM") as ps_o,
            ):
                for g in range(NG):
                    nkb = 8 * g + 8  # key blocks this group touches (both parities)
                    pu = [
                        ps_u.tile([128, 512], F32, tag=f"pu{d}", name=f"pu{d}")
                        for d in range(2)
                    ]
                    psums = ps_sum.tile([128, 512], F32, tag="psums")
                    for kb in range(nkb):
                        r = kb - 8 * g
                        qs = QS[r] if r >= 0 else 0
                        st = ps_sc.tile([128, 512], F32, tag="st")
                        for d in range(2):
                            nc.tensor.matmul(
                                st[:, qs:],
                                lhsT=kt[d][:, kb * 128 : (kb + 1) * 128],
                                rhs=qt[d][:, g * 512 + qs : (g + 1) * 512],
                                start=(d == 0), stop=(d == 1),
                            )
                        pt = ppool.tile([128, 512], BF16, tag="pt")
                        nc.scalar.activation(
                            pt[:, qs:], st[:, qs:],
                            mybir.ActivationFunctionType.Exp, scale=float(SCALE),
                        )
                        if r >= 0:
                            nc.vector.tensor_mul(
                                pt[:, qs:], pt[:, qs:], masks[r][:, qs:]
                            )
                        for d in range(2):
                            nc.tensor.matmul(
                                pu[d][:, qs:],
                                lhsT=zt[kb][:, d * 128 : (d + 1) * 128],
                                rhs=pt[:, qs:],
                                start=(kb == 0), stop=(kb == nkb - 1),
                            )
                        nc.tensor.matmul(
                            psums[:, qs:], lhsT=ones, rhs=pt[:, qs:],
                            start=(kb == 0), stop=(kb == nkb - 1),
                        )

                    ut = [
                        upool.tile([128, 512], BF16, tag=f"ut{d}", name=f"ut{d}")
                        for d in range(2)
                    ]
                    for d in range(2):
                        nc.scalar.copy(ut[d], pu[d])
                    # every psums row holds the same column sums; flip them to
                    # per-partition layout [128, 4] with 1/128-scaled matmuls.
                    ssb = stage.tile([128, 512], F32, tag="ssb")
                    nc.vector.tensor_copy(ssb, psums)
                    prc = ps_o.tile([128, 512], F32, tag="po", name="prc")
                    for s in range(4):
                        nc.tensor.matmul(
                            prc[:, s : s + 1],
                            lhsT=ssb[:, s * 128 : (s + 1) * 128],
                            rhs=inv128,
                            start=True, stop=True,
                        )
                    recg = stage.tile([128, 4], F32, tag="recg")
                    nc.vector.reciprocal(recg, prc[:, 0:4])

                    for s in range(4):
                        for eh in range(2):
                            esl = slice(eh * 512, (eh + 1) * 512)
                            po = ps_o.tile([128, 512], F32, tag="po")
                            for d in range(2):
                                nc.tensor.matmul(
                                    po,
                                    lhsT=ut[d][:, s * 128 : (s + 1) * 128],
                                    rhs=wvu_t[d][:, esl],
                                    start=(d == 0), stop=(d == 1),
                                )
                            ob = stage.tile([128, 512], BF16, tag="ob")
                            nc.vector.tensor_scalar_mul(ob, po, recg[:, s : s + 1])
                            nc.sync.dma_start(
                                out=o[(4 * g + s) * 128 : (4 * g + s + 1) * 128, esl],
                                in_=ob,
                            )
    nc.finalize()
    return nc


def _get_nc():
    if "nc" not in _CACHE:
        _CACHE["nc"] = _build_nc()
    return _CACHE["nc"]


def _host_masks(parity: int):
    """8 masks [128, 512] for the 8 most-recent key blocks of each group.

    mask[r][p, s*128 + t]: owned query block o = 2s+parity vs key rel-block r:
    o > r -> 1 ; o == r -> (t >= p) ; o < r -> 0.
    """
    import ml_dtypes

    t = np.arange(128)[None, :]
    p = np.arange(128)[:, None]
    tri = (t >= p).astype(np.float32)
    out = np.zeros((8, 128, 512), dtype=np.float32)
    for r in range(8):
        for s in range(4):
            osb = 2 * s + parity
            if osb > r:
                out[r][:, s * 128 : (s + 1) * 128] = 1.0
            elif osb == r:
                out[r][:, s * 128 : (s + 1) * 128] = tri
    return out.astype(ml_dtypes.bfloat16)


def _host_inputs(inputs, Wq, Wk, Wvdown, Wvup):
    import ml_dtypes

    bf16 = ml_dtypes.bfloat16
    inputs = np.asarray(inputs, dtype=np.float32)
    w = {
        "wq": np.ascontiguousarray(np.asarray(Wq, dtype=np.float32)).astype(bf16),
        "wk": np.ascontiguousarray(np.asarray(Wk, dtype=np.float32)).astype(bf16),
        "wvd": np.ascontiguousarray(np.asarray(Wvdown, dtype=np.float32)).astype(bf16),
        "wvu": np.ascontiguousarray(np.asarray(Wvup, dtype=np.float32)).astype(bf16),
    }
    xT_b = [np.ascontiguousarray(inputs[b].T).astype(bf16) for b in range(B)]
    masks = [_host_masks(0), _host_masks(1)]
    in_maps = []
    for core in range(NCORES):
        b, c = core // 2, core % 2
        xq = np.ascontiguousarray(
            xT_b[b].reshape(E, N // 128, 128)[:, c::2, :].reshape(E, QLOC)
        )
        in_maps.append(
            {"xT": xT_b[b], "xqT": xq, "mk": masks[c], **w}
        )
    return in_maps


def _assemble(results):
    out = np.empty((B, N, E), dtype=np.float32)
    for b in range(B):
        for c in range(2):
            ob = np.asarray(results[2 * b + c]["o"], dtype=np.float32)
            out[b].reshape(N // 128, 128, E)[c::2] = ob.reshape(QLOC // 128, 128, E)
    return out


def kernel(inputs, Wq, Wk, Wvdown, Wvup):
    from concourse.bass_utils import run_bass_kernel_spmd

    nc = _get_nc()
    in_maps = _host_inputs(inputs, Wq, Wk, Wvdown, Wvup)
    res = run_bass_kernel_spmd(nc, in_maps, core_ids=list(range(NCORES)))
    return _assemble(res.results)
